# revision 27
# baseline (speedup 1.0000x reference)
"""GATv2 3-layer GNN forward on 8 Trainium2 NeuronCores (Bass/Tile).

Sharding: edges (with self-loops) sorted by dst; core c owns dst nodes
[5000c, 5000(c+1)) so all segment reductions are core-local. Node tables
are replicated via per-layer AllGather (issued in 4 block-chunks so the
collective overlaps the producing layer's tail; the table row layout is
chunk-major so each chunk's output is one contiguous row range).

Layer 1 needs no node table at all: both endpoints' features are only
5/6-dim ([x0,xyz,1] for dst; + ea for src), shipped host-pre-gathered per
edge, and the PE computes z per edge as two short-stationary matmuls:
    psum_t = extd5^T @ WR1f5 + exts6^T @ WL1f6            (TensorE)
Messages are scattered as ex*z and corrected per dst node:
    sum_e ex*xl = sum_e ex*z - (xr[dst]-b1)*den - we*sum_e ex*ea
(bias1 is folded into the correction operand so the epilogue skips +bias).

Layers 2/3 gather xl[src] per BLOCK via two batched dma_gather calls
(SWDGE custom gather: ~1us fixed + 0.34ns/row, vs ~1.1us per 128-row
indirect_dma_start).  dma_gather takes int16 indices, so the 40960-row
table is addressed in a lo half (rows < 32768) and a hi half; each
block's edges are packed [self | lo tiles | hi tiles] host-side.  The
layer-3 table is padded to 128 cols (gather rows must be 256B-aligned).
    u = psum_t + gt                                       (VectorE)
    logits = sum_c att_c * prelu(u, 0.2)   (att applied raw, not |att|-
      fused into weights: the prelu-commute fold only forced an extra
      attrecip multiply in the epilogue)
    ex = exp(logits)  written by ScalarE DIRECTLY into the scatter tile's
      [D:D+H] columns (no ex copies on any engine)
    psum_blk += S^T @ [ex * gt | ex]                      (TensorE)
DVE ops keep all-unit-stride bf16 forms where possible (2x mode): the
att multiply uses a GB-tiled [128, GB*D] constant instead of a broadcast.

Block epilogue: h = tanh(num * (1/den) + bias); next-layer xl/xr via PE
transpose + matmul; xl staged to DRAM in 4 chunks, each AllGathered as
soon as its 10 blocks are done.  Pooling: one-hot matmul on local graph
ids, indirect-DMA scatter to [512,8], AllReduce, tiny linear head.
"""
import sys

for _p in ("/opt/trn_rl_repo",):
    if _p not in sys.path:
        sys.path.insert(0, _p)

import numpy as np

N = 40000
E = 500000
B = 512
NC = 8
NPC = N // NC            # nodes per core
BLK = 127                # real nodes per 128-row block (row 127 = ea/we slot)
NBLK = -(-NPC // BLK)    # blocks per core (40)
PADN = NBLK * 128        # padded node rows per core (5120)
HEADS = [(8, 32), (8, 16), (1, 8)]   # (H, C) per layer
DIMS = [h * c for h, c in HEADS]     # 256, 128, 8
# scatter width: layer 1 carries [ex*z | ex | ex*ea], layers 2/3 [ex*gt | ex]
WIDTHS = [DIMS[0] + 2 * HEADS[0][0], DIMS[1] + 3 * HEADS[1][0], DIMS[2] + 3 * HEADS[2][0]]
GBS = [4, 8, 16]         # edge tiles per elementwise batch, per layer
CHUNKS = 4               # AllGather chunks per layer boundary
BPC = NBLK // CHUNKS     # blocks per chunk
POOLPAD = 768
LOHALF = 32768           # int16 index limit: table rows < LOHALF via lo gather
TBW = 128                # padded table width for layer-3 gather (256B rows)

_CACHE = {}


def _padrow(n):
    """Node -> replicated-table row.  Layout is chunk-major so that each
    chunk's AllGather writes one contiguous row range:
    row = ((k*NC + c) * BPC + bo) * 128 + r,  k = b // BPC, bo = b % BPC."""
    c, nl = np.divmod(n, NPC)
    b, r = np.divmod(nl, BLK)
    k, bo = np.divmod(b, BPC)
    return ((k * NC + c) * BPC + bo) * 128 + r


def _wrap16(vals, ntiles):
    """int16 gather indices in SWDGE layout: position j -> [j%16, j//16],
    replicated to 128 partitions; padded to ntiles*128 positions with 0."""
    w = np.zeros((16, ntiles * 8), np.int16)
    j = np.arange(len(vals))
    w[j % 16, j // 16] = vals.astype(np.int16)
    return np.tile(w, (8, 1))


def _host_preprocess(x, edge_index, edge_attr, batch):
    src = np.asarray(edge_index[0], np.int64)
    dst = np.asarray(edge_index[1], np.int64)
    ea = np.asarray(edge_attr, np.float32).reshape(-1)

    # self loops, fill_value='mean' of incoming edge_attr
    deg = np.zeros(N, np.float32)
    np.add.at(deg, dst, np.float32(1.0))
    esum = np.zeros(N, np.float32)
    np.add.at(esum, dst, ea)
    loop_attr = np.where(deg > 0, esum / np.maximum(deg, 1.0), 0.0).astype(np.float32)
    src_f = np.concatenate([src, np.arange(N, dtype=np.int64)])
    dst_f = np.concatenate([dst, np.arange(N, dtype=np.int64)])
    ea_f = np.concatenate([ea, loop_attr]).astype(np.float32)

    order = np.argsort(dst_f, kind="stable")
    src_s, dst_s, ea_s = src_f[order], dst_f[order], ea_f[order]
    src_pad = _padrow(src_s).astype(np.int64)

    xf = np.asarray(x, np.float32)
    ext = np.concatenate([xf[:, :1], xf[:, 1:], np.ones((N, 1), np.float32)], 1)

    bounds = np.searchsorted(dst_s, np.arange(0, N + 1, 1))

    # classify per (c, b): self / lo-src / hi-src; shared tile counts = max
    # over cores so the SPMD program is identical on every core
    perm_cb = {}
    ntl = np.zeros(NBLK, np.int64)
    nth = np.zeros(NBLK, np.int64)
    for b in range(NBLK):
        for c in range(NC):
            lo = bounds[min(c * NPC + b * BLK, N)]
            hi = bounds[min(c * NPC + min((b + 1) * BLK, NPC), N)]
            idx = np.arange(lo, hi)
            is_self = src_s[lo:hi] == dst_s[lo:hi]
            self_idx = np.where(is_self)[0]
            extra, self_idx = self_idx[128:], self_idx[:128]
            norm_idx = np.concatenate([np.where(~is_self)[0], extra])
            is_lo = src_pad[lo + norm_idx] < LOHALF
            lo_idx = norm_idx[is_lo]
            hi_idx = norm_idx[~is_lo]
            perm_cb[c, b] = (lo + self_idx, lo + lo_idx, lo + hi_idx)
            ntl[b] = max(ntl[b], -(-len(lo_idx) // 128))
            nth[b] = max(nth[b], -(-len(hi_idx) // 128))
    nt_pb = 1 + ntl + nth
    T = int(nt_pb.sum())
    MAXNT = int(nt_pb.max())
    IDXW = 8 * (MAXNT - 1)

    st_blk = np.zeros((NC, NBLK, 128, MAXNT * 128), np.float32)
    stT_blk = np.zeros((NC, NBLK, 128, MAXNT * 128), np.float32)
    src32 = np.zeros((NC, 128, T), np.int32)
    ea_sb = np.zeros((NC, 128, T), np.float32)
    ext11 = np.zeros((NC, 11, T * 128), np.float32)
    idx16 = np.zeros((NC, 128, NBLK * IDXW), np.int16)
    t0 = 0
    for b in range(NBLK):
        for c in range(NC):
            selfs, los, his = perm_cb[c, b]
            idx = np.concatenate([selfs, los, his])
            pos = np.concatenate([
                np.arange(len(selfs)),
                128 + np.arange(len(los)),
                (1 + ntl[b]) * 128 + np.arange(len(his)),
            ])
            ti, pi = pos // 128, pos % 128
            dl = (dst_s[idx] - c * NPC - b * BLK).astype(np.int64)
            st_blk[c, b, dl, ti * 128 + pi] = 1.0
            st_blk[c, b, 127, ti * 128 + pi] = ea_s[idx]
            stT_blk[c, b, pi, ti * 128 + dl] = 1.0
            src32[c, pi, t0 + ti] = src_pad[idx]
            ea_sb[c, pi, t0 + ti] = ea_s[idx]
            ext11[c][0:5, (t0 + ti) * 128 + pi] = ext[dst_s[idx]].T
            ext11[c][5:10, (t0 + ti) * 128 + pi] = ext[src_s[idx]].T
            ext11[c][10, (t0 + ti) * 128 + pi] = ea_s[idx]
            if ntl[b]:
                idx16[c, :, b * IDXW: b * IDXW + ntl[b] * 8] = _wrap16(
                    src_pad[los], ntl[b])
            if nth[b]:
                o = b * IDXW + ntl[b] * 8
                idx16[c, :, o: o + nth[b] * 8] = _wrap16(
                    src_pad[his] - LOHALF, nth[b])
        t0 += nt_pb[b]

    # pooling metadata
    batch = np.asarray(batch, np.int64)
    gbase = np.array([batch[c * NPC] for c in range(NC)], np.int64)
    batchloc = np.full((NC, 128, NBLK), 200.0, np.float32)
    for c in range(NC):
        bl = batch[c * NPC:(c + 1) * NPC] - gbase[c]
        assert bl.max() < 127, "graph span exceeds 127 per core"
        for b in range(NBLK):
            nn = min((b + 1) * BLK, NPC) - b * BLK
            batchloc[c, :nn, b] = bl[b * BLK: b * BLK + nn]
    g_rows = np.zeros((NC, 128, 1), np.int32)
    for c in range(NC):
        rows = gbase[c] + np.arange(128)
        junk = B + 64 + np.arange(128)
        g_rows[c, :, 0] = np.where(rows < B, rows, junk)
    cnt = np.bincount(batch, minlength=B).astype(np.float32)
    rcnt = (1.0 / np.maximum(cnt, 1.0)).astype(np.float32)

    return dict(ntl=tuple(int(v) for v in ntl), nth=tuple(int(v) for v in nth),
                T=T, MAXNT=MAXNT, IDXW=IDXW,
                st_blk=st_blk, stT_blk=stT_blk, ea_sb=ea_sb, src32=src32,
                ext11=ext11, idx16=idx16,
                batchloc=batchloc, g_rows=g_rows, rcnt=rcnt)


def _host_weights(inp):
    out = {}
    # layer-1 input fusion: ext = [x0, xyz, 1]; h0 = ext @ M
    M = np.zeros((5, 7), np.float32)
    M[0, :4] = np.asarray(inp["w0"], np.float32)[0]
    M[1, 4] = M[2, 5] = M[3, 6] = 1.0
    M[4, :4] = np.asarray(inp["b0"], np.float32)
    wl1 = np.asarray(inp["wl1"], np.float32)
    wr1 = np.asarray(inp["wr1"], np.float32)
    we1 = np.asarray(inp["we1"], np.float32)
    out["W1c"] = np.concatenate([M @ wr1, M @ wl1, we1], 0).astype(np.float32)
    out["WR1f5"] = (M @ wr1).astype(np.float32)                          # [5,256]
    for i in (2, 3):
        out[f"WL{i}"] = np.asarray(inp[f"wl{i}"], np.float32)
        out[f"WR{i}"] = np.asarray(inp[f"wr{i}"], np.float32)
    for i, (H, C) in enumerate(HEADS, start=1):
        att = np.asarray(inp[f"att{i}"], np.float32).reshape(-1)     # [D]
        we = np.asarray(inp[f"we{i}"], np.float32)                   # [1, D]
        out[f"attB{i}"] = np.tile(att[None, :], (128, GBS[i - 1]))   # [128,GB*D]
        out[f"weaug{i}"] = np.tile(we, (1, NBLK)).astype(np.float32)
        out[f"weRep{i}"] = np.tile(we, (128, 1)).astype(np.float32)
        out[f"biasRep{i}"] = np.tile(np.asarray(inp[f"b{i}"], np.float32)[None, :],
                                     (128, 1))
    out["w4rep"] = np.tile(np.asarray(inp["w4"], np.float32)[:, 0][None, :], (128, 1))
    out["b4"] = float(np.asarray(inp["b4"], np.float32)[0])
    return out


def _build_x_inputs(x):
    x = np.asarray(x, np.float32)
    ext = np.concatenate([x[:, :1], x[:, 1:], np.ones((N, 1), np.float32)], 1)
    # own-core view in block order (block-local layout, not table layout)
    own = np.zeros((NC, PADN, 5), np.float32)
    n = np.arange(N)
    c, nl = np.divmod(n, NPC)
    b, r = np.divmod(nl, BLK)
    own[c, b * 128 + r] = ext
    xt6_own = np.ascontiguousarray(own.transpose(0, 2, 1))
    return xt6_own


def _build_program(ntl, nth, T, variant=""):
    import contextlib
    import concourse.bass as bass
    import concourse.bacc as bacc
    import concourse.mybir as mybir
    import concourse.tile as tile

    dt = mybir.dt
    f32 = dt.float32
    bf16 = dt.bfloat16
    i16 = dt.int16
    i32 = dt.int32
    Alu = mybir.AluOpType
    Act = mybir.ActivationFunctionType
    IOA = bass.IndirectOffsetOnAxis

    nt_pb = [1 + ntl[b] + nth[b] for b in range(NBLK)]
    MAXNT = max(nt_pb)
    IDXW = 8 * (MAXNT - 1)
    NOCC = "nocc" in variant
    OLDGATHER = "oldgather" in variant

    nc = bacc.Bacc("TRN2", target_bir_lowering=False, debug=False, num_devices=NC,
                   num_swdge_queues=4)

    ein = {}
    def EIN(name, shape, d=f32):
        ein[name] = nc.dram_tensor(name, list(shape), d, kind="ExternalInput")
        return ein[name]

    st_blk_d = EIN("st_blk", [NBLK, 128, MAXNT * 128], bf16)
    stT_blk_d = EIN("stT_blk", [NBLK, 128, MAXNT * 128], bf16)
    if OLDGATHER:
        src32_d = EIN("src32", [128, T], i32)
    else:
        idx16_d = EIN("idx16", [128, NBLK * IDXW], i16)
    ea_sb_d = EIN("ea_sb", [128, T])
    ext11_d = EIN("ext11", [11, T * 128], bf16)
    xt6_own_d = EIN("xt6_own", [5, PADN], bf16)
    W1c_d = EIN("W1c", [11, DIMS[0]], bf16)
    WR1f5_d = EIN("WR1f5", [5, DIMS[0]], bf16)
    WL2_d = EIN("WL2", [DIMS[0], DIMS[1]], bf16)
    WR2_d = EIN("WR2", [DIMS[0], DIMS[1]], bf16)
    WL3_d = EIN("WL3", [DIMS[1], DIMS[2]], bf16)
    WR3_d = EIN("WR3", [DIMS[1], DIMS[2]], bf16)
    weaug_d = [None] + [EIN(f"weaug{i}", [1, NBLK * DIMS[i - 1]], bf16)
                        for i in (2, 3)]
    weRep_d = [EIN(f"weRep{i}", [128, DIMS[i - 1]]) for i in (1, 2, 3)]
    attB_d = [EIN(f"attB{i}", [128, GBS[i - 1] * DIMS[i - 1]], bf16)
              for i in (1, 2, 3)]
    biasRep_d = [EIN(f"biasRep{i}", [128, DIMS[i - 1]]) for i in (1, 2, 3)]
    iota_d = EIN("iota_row", [128, 128], bf16)
    ident_d = EIN("ident", [128, 128], bf16)
    batchloc_d = EIN("batchloc", [128, NBLK], bf16)
    g_rows_d = EIN("g_rows", [128, 1], i32)
    rcnt_d = EIN("rcnt", [128, 4])
    w4rep_d = EIN("w4rep", [128, 8])
    b4_d = EIN("b4v", [128, 1])

    out_d = nc.dram_tensor("out", [B, 1], f32, kind="ExternalOutput")

    # layer-3 table padded to TBW cols so gather rows are 256B
    tables = [None,
              nc.dram_tensor("table2", [NC * PADN, DIMS[1]], bf16),
              nc.dram_tensor("table3", [NC * PADN, TBW], bf16)]
    stagec = [[nc.dram_tensor(f"stage{i}_{k}",
                              [BPC * 128, DIMS[1] if i == 2 else TBW], bf16)
               for k in range(CHUNKS)] for i in (2, 3)]
    pool_full = nc.dram_tensor("pool_full", [POOLPAD, 8], f32)
    pool_red = nc.dram_tensor("pool_red", [B, 8], f32)

    def issue_allgather(li, k):
        tab = tables[li + 1]
        w = tab.shape[1]
        r0 = k * NC * BPC * 128
        r1 = (k + 1) * NC * BPC * 128
        if NOCC:
            nc.sync.dma_start(tab[r0:r0 + BPC * 128, :], stagec[li][k][:])
        else:
            nc.gpsimd.collective_compute(
                "AllGather", mybir.AluOpType.bypass,
                replica_groups=[list(range(NC))],
                ins=[stagec[li][k].ap().opt()],
                outs=[tab.ap()[r0:r1, :].opt()],
            )

    with tile.TileContext(nc) as tc:
        ctx = contextlib.ExitStack()
        with ctx:
            consts = ctx.enter_context(tc.tile_pool(name="consts", bufs=1))
            meta = ctx.enter_context(tc.tile_pool(name="meta", bufs=1))
            xrp = ctx.enter_context(tc.tile_pool(name="xrp", bufs=1))
            stp = ctx.enter_context(tc.tile_pool(name="stp", bufs=4))
            gp = ctx.enter_context(tc.tile_pool(name="gp", bufs=4))
            gpg = ctx.enter_context(tc.tile_pool(name="gpg", bufs=6))
            sp = ctx.enter_context(tc.tile_pool(name="sp", bufs=4))
            ep = ctx.enter_context(tc.tile_pool(name="ep", bufs=3))
            pst = ctx.enter_context(tc.tile_pool(name="psum_t", bufs=2, space="PSUM"))
            psb = ctx.enter_context(tc.tile_pool(name="psum_blk", bufs=2, space="PSUM"))
            pse = ctx.enter_context(tc.tile_pool(name="psum_epi", bufs=1, space="PSUM"))
            chp = ctx.enter_context(tc.tile_pool(name="chunk", bufs=2))

            def load_const(dram, shape, d=f32):
                t = consts.tile(list(shape), d, tag=dram.name + "_c")
                nc.sync.dma_start(t[:], dram[:])
                return t
            iota_t = load_const(iota_d, [128, 128], bf16)
            ident_t = load_const(ident_d, [128, 128], bf16)
            W1c_t = load_const(W1c_d, [11, DIMS[0]], bf16)
            WR1f5_t = load_const(WR1f5_d, [5, DIMS[0]], bf16)
            WL2_t = [consts.tile([128, DIMS[1]], bf16, tag=f"wl2_{k}", name=f"wl2_{k}")
                     for k in range(2)]
            WR2_t = [consts.tile([128, DIMS[1]], bf16, tag=f"wr2_{k}", name=f"wr2_{k}")
                     for k in range(2)]
            for k in range(2):
                nc.sync.dma_start(WL2_t[k][:], WL2_d[k * 128:(k + 1) * 128, :])
                nc.sync.dma_start(WR2_t[k][:], WR2_d[k * 128:(k + 1) * 128, :])
            WL3_t = load_const(WL3_d, [128, DIMS[2]], bf16)
            WR3_t = load_const(WR3_d, [128, DIMS[2]], bf16)
            weRep_t = [load_const(weRep_d[i], [128, DIMS[i]]) for i in range(3)]
            attB_t = [load_const(attB_d[i], [128, GBS[i] * DIMS[i]], bf16)
                      for i in range(3)]
            biasRep_t = [load_const(biasRep_d[i], [128, DIMS[i]]) for i in range(3)]
            batchloc_t = load_const(batchloc_d, [128, NBLK], bf16)
            g_rows_t = load_const(g_rows_d, [128, 1], i32)
            rcnt_t = load_const(rcnt_d, [128, 4])
            w4rep_t = load_const(w4rep_d, [128, 8])
            b4_t = load_const(b4_d, [128, 1])
            if OLDGATHER:
                src_t = meta.tile([128, T], i32)
                nc.sync.dma_start(src_t[:], src32_d[:])
            else:
                idx_t = meta.tile([128, NBLK * IDXW], i16)
                nc.sync.dma_start(idx_t[:], idx16_d[:])
            ea_t = meta.tile([128, T], f32)
            nc.sync.dma_start(ea_t[:], ea_sb_d[:])

            xr_t = [xrp.tile([128, NBLK * DIMS[i]], bf16, tag=f"xr{i}", name=f"xr{i}")
                    for i in range(3)]
            for i in (1, 2):
                nc.sync.dma_start(xr_t[i][127:128, :], weaug_d[i][:])
            # xl of own nodes, kept in SBUF for the self-loop tiles of L2/L3
            # (row 127 zeroed so sblk's ea row contributes nothing); layer-3
            # slab padded to TBW-wide blocks to match the padded table
            xl_t = [None,
                    xrp.tile([128, NBLK * DIMS[1]], bf16, tag="xl1", name="xl1"),
                    xrp.tile([128, NBLK * TBW], bf16, tag="xl2", name="xl2")]
            for i in (1, 2):
                nc.gpsimd.memset(xl_t[i][:], 0.0)

            # ---- preamble: own xr1 (minus bias1: folds +bias into the
            # dst-correction so the L1 epilogue skips the bias add) ----
            CH = 16
            for ch in range(-(-NBLK // CH)):
                j0, j1 = ch * CH, min((ch + 1) * CH, NBLK)
                xchunk = chp.tile([5, CH * 128], bf16, tag="xchunk")
                nc.sync.dma_start(xchunk[:, :(j1 - j0) * 128],
                                  xt6_own_d[:, j0 * 128:j1 * 128])
                for j in range(j1 - j0):
                    b = j0 + j
                    pt = psb.tile([128, DIMS[0]], f32, tag="blk_ps", space="PSUM")
                    nc.tensor.matmul(pt[:], lhsT=xchunk[:, j * 128:(j + 1) * 128],
                                     rhs=WR1f5_t[:], start=True, stop=True)
                    D0 = DIMS[0]
                    nc.vector.tensor_tensor(
                        out=xr_t[0][0:127, b * D0:(b + 1) * D0],
                        in0=pt[0:127, :], in1=biasRep_t[0][0:127, :],
                        op=Alu.subtract)

            # ---- layers ----
            _gq = [0]
            pool_ps = psb.tile([128, 8], f32, tag="pool_ps", space="PSUM", bufs=1)
            for li in range(3):
                H, C = HEADS[li]
                D = DIMS[li]
                W = WIDTHS[li]
                GB = GBS[li]
                PSLOT = max(D, 8)
                table = tables[li]
                is_last = li == 2
                is_first = li == 0
                XLW = DIMS[li] if li < 2 else TBW   # xl_t/table col stride

                def issue_gathers(bb, tt0):
                    nl, nh = ntl[bb], nth[bb]
                    ntb = nt_pb[bb]
                    gtb = gpg.tile([128, (MAXNT - 1) * TBW], bf16,
                                   tag=f"gtb{li}", name=f"gtb{li}")
                    gtb3 = gtb[:, :(ntb - 1) * TBW].rearrange(
                        "p (t d) -> p t d", d=TBW)
                    if OLDGATHER:
                        for t in range(1, ntb):
                            nc.gpsimd.indirect_dma_start(
                                out=gtb3[:, t - 1, :], out_offset=None,
                                in_=table[:],
                                in_offset=IOA(ap=src_t[:, tt0 + t:tt0 + t + 1],
                                              axis=0))
                        return gtb3
                    # SWDGE gather caps at 1024 indices per call;
                    # round-robin the 4 SWDGE queues
                    GMAX = 8
                    regions = [(0, nl, table[:]),
                               (nl, nh, table[LOHALF:NC * PADN, :])]
                    for roff, rnt, rtab in regions:
                        for q0 in range(0, rnt, GMAX):
                            qn = min(GMAX, rnt - q0)
                            o = roff + q0
                            nc.gpsimd.dma_gather(
                                out_ap=gtb3[:, o:o + qn, :],
                                in_ap=rtab,
                                idxs_ap=idx_t[:, bb * IDXW + o * 8:
                                              bb * IDXW + (o + qn) * 8],
                                num_idxs=qn * 128,
                                num_idxs_reg=qn * 128,
                                elem_size=TBW,
                                queue_num=_gq[0] % 4)
                            _gq[0] += 1
                    return gtb3

                def run_epilogue(b, pblk):
                    den = sp.tile([128, H], f32, tag="den")
                    nc.vector.tensor_scalar_add(den[:], pblk[:, D:D + H], 1e-30)
                    rden = sp.tile([128, H], f32, tag="rden")
                    nc.vector.reciprocal(rden[:], den[:])
                    hr = ep.tile([128, D], f32, tag="hr")
                    # num = pblk - xr[dst]*cden - we*cexea; for L1 the psum-based
                    # messages cover all edges (cden = den), for L2/L3 only the
                    # self tile (selfden / selfexea scatter columns)
                    if is_first:
                        cden = den
                        exea_src = pblk[:, D + H:D + 2 * H]
                    else:
                        cden = sp.tile([128, H], f32, tag="sden")
                        nc.vector.tensor_copy(cden[:], pblk[:, D + H:D + 2 * H])
                        exea_src = pblk[:, D + 2 * H:D + 3 * H]
                    exea = sp.tile([128, H], f32, tag="exea")
                    nc.vector.tensor_copy(exea[:], exea_src)
                    meng = nc.gpsimd if is_first else nc.vector
                    meng.tensor_tensor(
                        out=hr[:].rearrange("p (h c) -> p h c", h=H),
                        in0=xr_t[li][:, b * D:(b + 1) * D].rearrange(
                            "p (h c) -> p h c", h=H),
                        in1=cden[:].unsqueeze(2).to_broadcast([128, H, C]),
                        op=Alu.mult)
                    num = ep.tile([128, D], f32, tag="num")
                    nc.vector.tensor_tensor(out=num[:], in0=pblk[:, 0:D],
                                            in1=hr[:], op=Alu.subtract)
                    hr2 = ep.tile([128, D], f32, tag="hr2")
                    meng.tensor_tensor(
                        out=hr2[:].rearrange("p (h c) -> p h c", h=H),
                        in0=weRep_t[li][:].rearrange("p (h c) -> p h c", h=H),
                        in1=exea[:].unsqueeze(2).to_broadcast([128, H, C]),
                        op=Alu.mult)
                    nc.vector.tensor_tensor(out=num[:], in0=num[:],
                                            in1=hr2[:], op=Alu.subtract)
                    nc.vector.tensor_tensor(
                        out=hr[:].rearrange("p (h c) -> p h c", h=H),
                        in0=num[:].rearrange("p (h c) -> p h c", h=H),
                        in1=rden[:].unsqueeze(2).to_broadcast([128, H, C]),
                        op=Alu.mult)
                    if not is_first:
                        nc.vector.tensor_tensor(out=hr[:], in0=hr[:],
                                                in1=biasRep_t[li][:], op=Alu.add)
                    h = ep.tile([128, D], bf16, tag="h_blk")
                    nc.scalar.activation(h[:], hr[:], Act.Tanh)

                    if not is_last:
                        D2 = DIMS[li + 1]
                        XLW2 = DIMS[li + 1] if li + 1 < 2 else TBW
                        WLn = [WL2_t[0], WL2_t[1]] if li == 0 else [WL3_t]
                        WRn = [WR2_t[0], WR2_t[1]] if li == 0 else [WR3_t]
                        nk = D // 128
                        hT = []
                        for k in range(nk):
                            tp = pse.tile([128, 128], bf16, tag="epi_ps", space="PSUM")
                            nc.tensor.transpose(tp[:], h[:, k * 128:(k + 1) * 128],
                                                ident_t[:])
                            hTk = ep.tile([128, 128], bf16, tag=f"hT{k}")
                            nc.vector.tensor_copy(hTk[:], tp[:])
                            hT.append(hTk)
                        pxl = pse.tile([128, D2], f32, tag="epi_ps", space="PSUM")
                        for k in range(nk):
                            nc.tensor.matmul(pxl[:], lhsT=hT[k][:], rhs=WLn[k][:],
                                             start=(k == 0), stop=(k == nk - 1))
                        nc.vector.tensor_copy(
                            xl_t[li + 1][0:127, b * XLW2:b * XLW2 + D2],
                            pxl[0:127, :])
                        kc, bo = divmod(b, BPC)
                        nc.sync.dma_start(
                            stagec[li][kc][bo * 128:(bo + 1) * 128, :],
                            xl_t[li + 1][:, b * XLW2:(b + 1) * XLW2])
                        pxr = pse.tile([128, D2], f32, tag="epi_ps", space="PSUM")
                        for k in range(nk):
                            nc.tensor.matmul(pxr[:], lhsT=hT[k][:], rhs=WRn[k][:],
                                             start=(k == 0), stop=(k == nk - 1))
                        nc.vector.tensor_copy(
                            xr_t[li + 1][0:127, b * D2:(b + 1) * D2], pxr[0:127, :])
                        if bo == BPC - 1:
                            issue_allgather(li, kc)
                    else:
                        Sg = stp.tile([128, 128], bf16, tag="sg_tile")
                        nc.vector.tensor_tensor(
                            out=Sg[:],
                            in0=batchloc_t[:, b:b + 1].to_broadcast([128, 128]),
                            in1=iota_t[:], op=Alu.is_equal)
                        nc.tensor.matmul(pool_ps[:], lhsT=Sg[:], rhs=h[:],
                                         start=(b == 0), stop=(b == NBLK - 1))

                pending = None
                LA = 4
                gtbs = {}
                if not is_first:
                    tt = 0
                    for bb in range(NBLK):
                        if bb < LA:
                            gtbs[bb] = issue_gathers(bb, tt)
                        tt += nt_pb[bb]
                t0 = 0
                tla = sum(nt_pb[:LA]) if NBLK > LA else 0
                for b in range(NBLK):
                    nl, nh = ntl[b], nth[b]
                    nt = nt_pb[b]
                    pblk = psb.tile([128, W], f32, tag="blk_ps", space="PSUM")
                    stT = stp.tile([128, MAXNT * 128], bf16, tag="stT_blk")
                    nc.sync.dma_start(stT[:, :nt * 128],
                                      stT_blk_d[b, :, :nt * 128])
                    if is_first:
                        extb = chp.tile([11, MAXNT * 128], bf16, tag="extb")
                        nc.sync.dma_start(extb[:, :nt * 128],
                                          ext11_d[:, t0 * 128:(t0 + nt) * 128])
                    else:
                        if b + LA < NBLK:
                            gtbs[b + LA] = issue_gathers(b + LA, tla)
                            tla += nt_pb[b + LA]
                        sblk = stp.tile([128, MAXNT * 128], bf16, tag="st_blk")
                        nc.sync.dma_start(sblk[:, :nt * 128],
                                          st_blk_d[b, :, :nt * 128])
                        gtb3 = gtbs.pop(b)
                        # ---- self-loop tile 0: xl is block-local, no gather.
                        # Scatter carries [ex*z | ex | ex | ex*ea]; epilogue
                        # corrections remove the xr and ea*we parts.
                        ptile = pst.tile([128, GB, PSLOT], f32, tag="t_ps",
                                         space="PSUM")
                        nc.tensor.matmul(
                            ptile[:, 0, 0:D], lhsT=sblk[:, 0:128],
                            rhs=xr_t[li][:, b * D:(b + 1) * D],
                            start=True, stop=False)
                        nc.tensor.matmul(
                            ptile[:, 0, 0:D], lhsT=sblk[:, 0:128],
                            rhs=xl_t[li][:, b * XLW:b * XLW + D],
                            start=False, stop=True)
                        up = sp.tile([128, GB * D], bf16, tag="up_t")
                        nc.scalar.activation(
                            up[:, :D].rearrange("p (g d) -> p g d", g=1),
                            ptile[:, 0:1, 0:D], Act.Prelu, alpha=0.2)
                        v = sp.tile([128, GB * D], bf16, tag="v_t")
                        nc.vector.tensor_tensor(
                            out=v[:, :D], in0=up[:, :D],
                            in1=attB_t[li][:, :D], op=Alu.mult)
                        lg = sp.tile([128, GB * H], bf16, tag="lg")
                        with nc.allow_low_precision(reason="bf16 logits"):
                            nc.vector.tensor_reduce(
                                out=lg[:, :H],
                                in_=v[:, :D].rearrange("p (gh c) -> p gh c", c=C),
                                axis=mybir.AxisListType.X, op=Alu.add)
                        yt = gp.tile([128, GB, W], bf16, tag="y_tile")
                        nc.scalar.activation(
                            yt[:, 0:1, D:D + H],
                            lg[:, :H].rearrange("p (g h) -> p g h", g=1),
                            Act.Exp)
                        exv = yt[:, 0:1, D:D + H]
                        nc.vector.tensor_tensor(
                            out=yt[:, 0:1, 0:D].rearrange("p g (h c) -> p g h c",
                                                          h=H),
                            in0=ptile[:, 0:1, 0:D].rearrange(
                                "p g (h c) -> p g h c", h=H),
                            in1=exv.unsqueeze(3).to_broadcast([128, 1, H, C]),
                            op=Alu.mult)
                        nc.scalar.activation(
                            yt[:, 0:1, D + H:D + 2 * H],
                            lg[:, :H].rearrange("p (g h) -> p g h", g=1),
                            Act.Exp)
                        nc.scalar.activation(
                            yt[:, 0:1, D + 2 * H:D + 3 * H], exv,
                            Act.Copy, scale=ea_t[:, t0:t0 + 1])
                        nc.tensor.matmul(pblk[:], lhsT=stT[:, 0:128],
                                         rhs=yt[:, 0, :],
                                         start=True, stop=(nt == 1))
                    for g0 in range(0 if is_first else 1, nt, GB):
                        gs = min(GB, nt - g0)
                        ptile = pst.tile([128, GB, PSLOT], f32, tag="t_ps",
                                         space="PSUM")
                        for i in range(gs):
                            t = g0 + i
                            if is_first:
                                nc.tensor.matmul(
                                    ptile[:, i, 0:D],
                                    lhsT=extb[:, t * 128:(t + 1) * 128],
                                    rhs=W1c_t[:], start=True, stop=True)
                            else:
                                nc.tensor.matmul(
                                    ptile[:, i, 0:D],
                                    lhsT=sblk[:, t * 128:(t + 1) * 128],
                                    rhs=xr_t[li][:, b * D:(b + 1) * D],
                                    start=True, stop=False)
                                nc.tensor.matmul(
                                    ptile[:, i, 0:D],
                                    lhsT=ident_t[:],
                                    rhs=gtb3[:, t - 1, 0:D],
                                    start=False, stop=True)
                        gsl = gtb3[:, g0 - 1:g0 - 1 + gs, 0:D] if not is_first \
                            else None
                        zsrc = ptile[:, 0:gs, 0:D]
                        up = sp.tile([128, GB * D], bf16, tag="up_t")
                        nc.scalar.activation(
                            up[:, :gs * D].rearrange("p (g d) -> p g d", g=gs),
                            zsrc, Act.Prelu, alpha=0.2)
                        v = sp.tile([128, GB * D], bf16, tag="v_t")
                        nc.vector.tensor_tensor(
                            out=v[:, :gs * D], in0=up[:, :gs * D],
                            in1=attB_t[li][:, :gs * D], op=Alu.mult)
                        lg = sp.tile([128, GB * H], bf16, tag="lg")
                        with nc.allow_low_precision(reason="bf16 logits"):
                            nc.vector.tensor_reduce(
                                out=lg[:, :gs * H],
                                in_=v[:, :gs * D].rearrange("p (gh c) -> p gh c",
                                                            c=C),
                                axis=mybir.AxisListType.X, op=Alu.add)
                        yt = gp.tile([128, GB, W], bf16, tag="y_tile")
                        nc.scalar.activation(
                            yt[:, 0:gs, D:D + H],
                            lg[:, :gs * H].rearrange("p (g h) -> p g h", g=gs),
                            Act.Exp)
                        exv = yt[:, 0:gs, D:D + H]
                        nc.vector.tensor_tensor(
                            out=yt[:, 0:gs, 0:D].rearrange("p g (h c) -> p g h c",
                                                           h=H),
                            in0=(zsrc if is_first else gsl).rearrange(
                                     "p g (h c) -> p g h c", h=H),
                            in1=exv.unsqueeze(3).to_broadcast([128, gs, H, C]),
                            op=Alu.mult)
                        if is_first:
                            nc.gpsimd.tensor_tensor(
                                out=yt[:, 0:gs, D + H:D + 2 * H],
                                in0=exv,
                                in1=ea_t[:, t0 + g0:t0 + g0 + gs].unsqueeze(2)
                                    .to_broadcast([128, gs, H]),
                                op=Alu.mult)
                        for i in range(gs):
                            nc.tensor.matmul(
                                pblk[:] if is_first else pblk[:, 0:D + H],
                                lhsT=stT[:, (g0 + i) * 128:(g0 + i + 1) * 128],
                                rhs=yt[:, i, 0:W] if is_first else yt[:, i, 0:D + H],
                                start=(is_first and g0 == 0 and i == 0),
                                stop=(g0 + i == nt - 1))
                    t0 += nt
                    if pending is not None:
                        run_epilogue(*pending)
                    pending = (b, pblk)
                if pending is not None:
                    run_epilogue(*pending)
            # ---- pooling + head ----
            pool_sb = ep.tile([128, 8], f32, tag="pool_sb")
            nc.vector.tensor_copy(pool_sb[:], pool_ps[:])
            zero8 = consts.tile([128, 8], f32, tag="zero8")
            nc.gpsimd.memset(zero8[:], 0.0)
            for i in range(POOLPAD // 128):
                nc.sync.dma_start(pool_full[i * 128:(i + 1) * 128, :], zero8[:])
            nc.gpsimd.indirect_dma_start(
                out=pool_full[:], out_offset=IOA(ap=g_rows_t[:, :1], axis=0),
                in_=pool_sb[:], in_offset=None)
            if NOCC:
                nc.sync.dma_start(pool_red[:], pool_full[0:B, :])
            else:
                nc.gpsimd.collective_compute(
                    "AllReduce", mybir.AluOpType.add,
                    replica_groups=[list(range(NC))],
                    ins=[pool_full.ap()[0:B, :].opt()], outs=[pool_red.ap().opt()])
            for i in range(B // 128):
                pt = ep.tile([128, 8], f32, tag="head_in")
                nc.sync.dma_start(pt[:], pool_red[i * 128:(i + 1) * 128, :])
                pw = ep.tile([128, 8], f32, tag="head_w")
                nc.vector.tensor_tensor(out=pw[:], in0=pt[:], in1=w4rep_t[:],
                                        op=Alu.mult)
                hred = ep.tile([128, 1], f32, tag="head_red")
                nc.vector.tensor_reduce(out=hred[:], in_=pw[:],
                                        axis=mybir.AxisListType.X, op=Alu.add)
                nc.vector.tensor_tensor(out=hred[:], in0=hred[:],
                                        in1=rcnt_t[:, i:i + 1], op=Alu.mult)
                nc.vector.tensor_tensor(out=hred[:], in0=hred[:], in1=b4_t[:],
                                        op=Alu.add)
                nc.sync.dma_start(out_d[i * 128:(i + 1) * 128, :], hred[:])

    nc.compile()
    nc._kernel_input_names = set(ein)
    return nc


def _get_program(inputs):
    import os
    variant = os.environ.get("KVARIANT", "")
    pre = _host_preprocess(inputs["x"], inputs["edge_index"], inputs["edge_attr"],
                           inputs["batch"])
    key = (pre["ntl"], pre["nth"], variant)
    if key not in _CACHE:
        _CACHE[key] = _build_program(pre["ntl"], pre["nth"], pre["T"],
                                     variant=variant)
    return _CACHE[key], pre


def _make_in_maps(inputs, pre):
    import ml_dtypes
    bf16 = ml_dtypes.bfloat16
    wts = _host_weights(inputs)
    xt6_own = _build_x_inputs(inputs["x"])
    iota = np.tile(np.arange(128, dtype=np.float32), (128, 1))
    ident = np.eye(128, dtype=np.float32)
    in_maps = []
    for c in range(NC):
        m = dict(
            st_blk=pre["st_blk"][c].astype(bf16),
            stT_blk=pre["stT_blk"][c].astype(bf16),
            idx16=pre["idx16"][c],
            src32=pre["src32"][c],
            ea_sb=pre["ea_sb"][c],
            ext11=pre["ext11"][c].astype(bf16),
            xt6_own=xt6_own[c].astype(bf16),
            W1c=wts["W1c"].astype(bf16), WR1f5=wts["WR1f5"].astype(bf16),
            WL2=wts["WL2"].astype(bf16), WR2=wts["WR2"].astype(bf16),
            WL3=wts["WL3"].astype(bf16), WR3=wts["WR3"].astype(bf16),
            weRep1=wts["weRep1"], weRep2=wts["weRep2"], weRep3=wts["weRep3"],
            iota_row=iota.astype(bf16), ident=ident.astype(bf16),
            batchloc=pre["batchloc"][c].astype(bf16), g_rows=pre["g_rows"][c],
            rcnt=np.ascontiguousarray(pre["rcnt"].reshape(4, 128).T),
            w4rep=wts["w4rep"], b4v=np.full((128, 1), wts["b4"], np.float32),
        )
        for i in (1, 2, 3):
            m[f"weaug{i}"] = wts[f"weaug{i}"].astype(bf16)
            m[f"attB{i}"] = wts[f"attB{i}"].astype(bf16)
            m[f"biasRep{i}"] = wts[f"biasRep{i}"]
        in_maps.append(m)
    return in_maps


def kernel(**inputs):
    from concourse.bass_utils import run_bass_kernel_spmd
    nc, pre = _get_program(inputs)
    in_maps = _make_in_maps(inputs, pre)
    names = getattr(nc, "_kernel_input_names", None)
    if names is not None:
        in_maps = [{k: v for k, v in m.items() if k in names} for m in in_maps]
    res = run_bass_kernel_spmd(nc, in_maps, core_ids=list(range(NC)))
    return np.asarray(res.results[0]["out"], np.float32)


# revision 28
# speedup vs baseline: 1.0709x; 1.0709x over previous
"""GATv2 3-layer GNN forward on 8 Trainium2 NeuronCores (Bass/Tile).

Sharding: edges (with self-loops) sorted by dst; core c owns dst nodes
[5000c, 5000(c+1)) so all segment reductions are core-local. Node tables
are replicated via per-layer AllGather (issued in 4 block-chunks so the
collective overlaps the producing layer's tail; the table row layout is
chunk-major so each chunk's output is one contiguous row range).

Layer 1 needs no node table at all: both endpoints' features are only
5/6-dim ([x0,xyz,1] for dst; + ea for src), shipped host-pre-gathered per
edge, and the PE computes z per edge as two short-stationary matmuls:
    psum_t = extd5^T @ WR1f5 + exts6^T @ WL1f6            (TensorE)
Messages are scattered as ex*z and corrected per dst node:
    sum_e ex*xl = sum_e ex*z - (xr[dst]-b1)*den - we*sum_e ex*ea
(bias1 is folded into the correction operand so the epilogue skips +bias).

Layers 2/3 gather xl[src] per BLOCK via two batched dma_gather calls
(SWDGE custom gather: ~1us fixed + 0.34ns/row, vs ~1.1us per 128-row
indirect_dma_start).  dma_gather takes int16 indices, so the 40960-row
table is addressed in a lo half (rows < 32768) and a hi half; each
block's edges are packed [self | lo tiles | hi tiles] host-side.  The
layer-3 table is padded to 128 cols (gather rows must be 256B-aligned).
    u = psum_t + gt                                       (VectorE)
    logits = sum_c att_c * prelu(u, 0.2)   (att applied raw, not |att|-
      fused into weights: the prelu-commute fold only forced an extra
      attrecip multiply in the epilogue)
    ex = exp(logits)  written by ScalarE DIRECTLY into the scatter tile's
      [D:D+H] columns (no ex copies on any engine)
    psum_blk += S^T @ [ex * gt | ex]                      (TensorE)
DVE ops keep all-unit-stride bf16 forms where possible (2x mode): the
att multiply uses a GB-tiled [128, GB*D] constant instead of a broadcast.

Block epilogue: h = tanh(num * (1/den) + bias); next-layer xl/xr via PE
transpose + matmul; xl staged to DRAM in 4 chunks, each AllGathered as
soon as its 10 blocks are done.  Pooling: one-hot matmul on local graph
ids, indirect-DMA scatter to [512,8], AllReduce, tiny linear head.
"""
import sys

for _p in ("/opt/trn_rl_repo",):
    if _p not in sys.path:
        sys.path.insert(0, _p)

import numpy as np

N = 40000
E = 500000
B = 512
NC = 8
NPC = N // NC            # nodes per core
BLK = 127                # real nodes per 128-row block (row 127 = ea/we slot)
NBLK = -(-NPC // BLK)    # blocks per core (40)
PADN = NBLK * 128        # padded node rows per core (5120)
HEADS = [(8, 32), (8, 16), (1, 8)]   # (H, C) per layer
DIMS = [h * c for h, c in HEADS]     # 256, 128, 8
# scatter width: layer 1 carries [ex*z | ex | ex*ea], layers 2/3 [ex*gt | ex]
WIDTHS = [DIMS[0] + 2 * HEADS[0][0], DIMS[1] + 3 * HEADS[1][0], DIMS[2] + 3 * HEADS[2][0]]
GBS = [4, 8, 16]         # edge tiles per elementwise batch, per layer
CHUNKS = 4               # AllGather chunks per layer boundary
BPC = NBLK // CHUNKS     # blocks per chunk
POOLPAD = 768
LOHALF = 32768           # int16 index limit: table rows < LOHALF via lo gather
TBW = 128                # padded table width for layer-3 gather (256B rows)

_CACHE = {}


def _padrow(n):
    """Node -> replicated-table row.  Layout is chunk-major so that each
    chunk's AllGather writes one contiguous row range:
    row = ((k*NC + c) * BPC + bo) * 128 + r,  k = b // BPC, bo = b % BPC."""
    c, nl = np.divmod(n, NPC)
    b, r = np.divmod(nl, BLK)
    k, bo = np.divmod(b, BPC)
    return ((k * NC + c) * BPC + bo) * 128 + r


def _wrap16(vals, ntiles):
    """int16 gather indices in SWDGE layout: position j -> [j%16, j//16],
    replicated to 128 partitions; padded to ntiles*128 positions with 0."""
    w = np.zeros((16, ntiles * 8), np.int16)
    j = np.arange(len(vals))
    w[j % 16, j // 16] = vals.astype(np.int16)
    return np.tile(w, (8, 1))


def _host_preprocess(x, edge_index, edge_attr, batch):
    src = np.asarray(edge_index[0], np.int64)
    dst = np.asarray(edge_index[1], np.int64)
    ea = np.asarray(edge_attr, np.float32).reshape(-1)

    # self loops, fill_value='mean' of incoming edge_attr
    deg = np.zeros(N, np.float32)
    np.add.at(deg, dst, np.float32(1.0))
    esum = np.zeros(N, np.float32)
    np.add.at(esum, dst, ea)
    loop_attr = np.where(deg > 0, esum / np.maximum(deg, 1.0), 0.0).astype(np.float32)
    src_f = np.concatenate([src, np.arange(N, dtype=np.int64)])
    dst_f = np.concatenate([dst, np.arange(N, dtype=np.int64)])
    ea_f = np.concatenate([ea, loop_attr]).astype(np.float32)

    order = np.argsort(dst_f, kind="stable")
    src_s, dst_s, ea_s = src_f[order], dst_f[order], ea_f[order]
    src_pad = _padrow(src_s).astype(np.int64)

    xf = np.asarray(x, np.float32)
    ext = np.concatenate([xf[:, :1], xf[:, 1:], np.ones((N, 1), np.float32)], 1)

    bounds = np.searchsorted(dst_s, np.arange(0, N + 1, 1))

    # classify per (c, b): self / lo-src / hi-src; shared tile counts = max
    # over cores so the SPMD program is identical on every core
    perm_cb = {}
    ntl = np.zeros(NBLK, np.int64)
    nth = np.zeros(NBLK, np.int64)
    for b in range(NBLK):
        for c in range(NC):
            lo = bounds[min(c * NPC + b * BLK, N)]
            hi = bounds[min(c * NPC + min((b + 1) * BLK, NPC), N)]
            idx = np.arange(lo, hi)
            is_self = src_s[lo:hi] == dst_s[lo:hi]
            self_idx = np.where(is_self)[0]
            extra, self_idx = self_idx[128:], self_idx[:128]
            norm_idx = np.concatenate([np.where(~is_self)[0], extra])
            is_lo = src_pad[lo + norm_idx] < LOHALF
            lo_idx = norm_idx[is_lo]
            hi_idx = norm_idx[~is_lo]
            perm_cb[c, b] = (lo + self_idx, lo + lo_idx, lo + hi_idx)
            ntl[b] = max(ntl[b], -(-len(lo_idx) // 128))
            nth[b] = max(nth[b], -(-len(hi_idx) // 128))
    nt_pb = 1 + ntl + nth
    T = int(nt_pb.sum())
    MAXNT = int(nt_pb.max())
    IDXW = 8 * (MAXNT - 1)

    st_blk = np.zeros((NC, NBLK, 128, MAXNT * 128), np.float32)
    stT_blk = np.zeros((NC, NBLK, 128, MAXNT * 128), np.float32)
    src32 = np.zeros((NC, 128, T), np.int32)
    ea_sb = np.zeros((NC, 128, T), np.float32)
    ext11 = np.zeros((NC, 11, T * 128), np.float32)
    idx16 = np.zeros((NC, 128, NBLK * IDXW), np.int16)
    t0 = 0
    for b in range(NBLK):
        for c in range(NC):
            selfs, los, his = perm_cb[c, b]
            idx = np.concatenate([selfs, los, his])
            pos = np.concatenate([
                np.arange(len(selfs)),
                128 + np.arange(len(los)),
                (1 + ntl[b]) * 128 + np.arange(len(his)),
            ])
            ti, pi = pos // 128, pos % 128
            dl = (dst_s[idx] - c * NPC - b * BLK).astype(np.int64)
            st_blk[c, b, dl, ti * 128 + pi] = 1.0
            st_blk[c, b, 127, ti * 128 + pi] = ea_s[idx]
            stT_blk[c, b, pi, ti * 128 + dl] = 1.0
            src32[c, pi, t0 + ti] = src_pad[idx]
            ea_sb[c, pi, t0 + ti] = ea_s[idx]
            ext11[c][0:5, (t0 + ti) * 128 + pi] = ext[dst_s[idx]].T
            ext11[c][5:10, (t0 + ti) * 128 + pi] = ext[src_s[idx]].T
            ext11[c][10, (t0 + ti) * 128 + pi] = ea_s[idx]
            if ntl[b]:
                idx16[c, :, b * IDXW: b * IDXW + ntl[b] * 8] = _wrap16(
                    src_pad[los], ntl[b])
            if nth[b]:
                o = b * IDXW + ntl[b] * 8
                idx16[c, :, o: o + nth[b] * 8] = _wrap16(
                    src_pad[his] - LOHALF, nth[b])
        t0 += nt_pb[b]

    # pooling metadata
    batch = np.asarray(batch, np.int64)
    gbase = np.array([batch[c * NPC] for c in range(NC)], np.int64)
    batchloc = np.full((NC, 128, NBLK), 200.0, np.float32)
    for c in range(NC):
        bl = batch[c * NPC:(c + 1) * NPC] - gbase[c]
        assert bl.max() < 127, "graph span exceeds 127 per core"
        for b in range(NBLK):
            nn = min((b + 1) * BLK, NPC) - b * BLK
            batchloc[c, :nn, b] = bl[b * BLK: b * BLK + nn]
    g_rows = np.zeros((NC, 128, 1), np.int32)
    for c in range(NC):
        rows = gbase[c] + np.arange(128)
        junk = B + 64 + np.arange(128)
        g_rows[c, :, 0] = np.where(rows < B, rows, junk)
    cnt = np.bincount(batch, minlength=B).astype(np.float32)
    rcnt = (1.0 / np.maximum(cnt, 1.0)).astype(np.float32)

    return dict(ntl=tuple(int(v) for v in ntl), nth=tuple(int(v) for v in nth),
                T=T, MAXNT=MAXNT, IDXW=IDXW,
                st_blk=st_blk, stT_blk=stT_blk, ea_sb=ea_sb, src32=src32,
                ext11=ext11, idx16=idx16,
                batchloc=batchloc, g_rows=g_rows, rcnt=rcnt)


def _host_weights(inp):
    out = {}
    # layer-1 input fusion: ext = [x0, xyz, 1]; h0 = ext @ M
    M = np.zeros((5, 7), np.float32)
    M[0, :4] = np.asarray(inp["w0"], np.float32)[0]
    M[1, 4] = M[2, 5] = M[3, 6] = 1.0
    M[4, :4] = np.asarray(inp["b0"], np.float32)
    wl1 = np.asarray(inp["wl1"], np.float32)
    wr1 = np.asarray(inp["wr1"], np.float32)
    we1 = np.asarray(inp["we1"], np.float32)
    out["W1c"] = np.concatenate([M @ wr1, M @ wl1, we1], 0).astype(np.float32)
    out["WR1f5"] = (M @ wr1).astype(np.float32)                          # [5,256]
    for i in (2, 3):
        out[f"WL{i}"] = np.asarray(inp[f"wl{i}"], np.float32)
        out[f"WR{i}"] = np.asarray(inp[f"wr{i}"], np.float32)
    for i, (H, C) in enumerate(HEADS, start=1):
        att = np.asarray(inp[f"att{i}"], np.float32).reshape(-1)     # [D]
        we = np.asarray(inp[f"we{i}"], np.float32)                   # [1, D]
        out[f"attB{i}"] = np.tile(att[None, :], (128, GBS[i - 1]))   # [128,GB*D]
        out[f"weaug{i}"] = np.tile(we, (1, NBLK)).astype(np.float32)
        out[f"weRep{i}"] = np.tile(we, (128, 1)).astype(np.float32)
        out[f"biasRep{i}"] = np.tile(np.asarray(inp[f"b{i}"], np.float32)[None, :],
                                     (128, 1))
    out["w4rep"] = np.tile(np.asarray(inp["w4"], np.float32)[:, 0][None, :], (128, 1))
    out["b4"] = float(np.asarray(inp["b4"], np.float32)[0])
    return out


def _build_x_inputs(x):
    x = np.asarray(x, np.float32)
    ext = np.concatenate([x[:, :1], x[:, 1:], np.ones((N, 1), np.float32)], 1)
    # own-core view in block order (block-local layout, not table layout)
    own = np.zeros((NC, PADN, 5), np.float32)
    n = np.arange(N)
    c, nl = np.divmod(n, NPC)
    b, r = np.divmod(nl, BLK)
    own[c, b * 128 + r] = ext
    xt6_own = np.ascontiguousarray(own.transpose(0, 2, 1))
    return xt6_own


def _build_program(ntl, nth, T, variant=""):
    import contextlib
    import concourse.bass as bass
    import concourse.bacc as bacc
    import concourse.mybir as mybir
    import concourse.tile as tile

    dt = mybir.dt
    f32 = dt.float32
    bf16 = dt.bfloat16
    i16 = dt.int16
    i32 = dt.int32
    Alu = mybir.AluOpType
    Act = mybir.ActivationFunctionType
    IOA = bass.IndirectOffsetOnAxis

    nt_pb = [1 + ntl[b] + nth[b] for b in range(NBLK)]
    MAXNT = max(nt_pb)
    IDXW = 8 * (MAXNT - 1)
    NOCC = "nocc" in variant
    OLDGATHER = "oldgather" in variant

    nc = bacc.Bacc("TRN2", target_bir_lowering=False, debug=False, num_devices=NC,
                   num_swdge_queues=4)

    ein = {}
    def EIN(name, shape, d=f32):
        ein[name] = nc.dram_tensor(name, list(shape), d, kind="ExternalInput")
        return ein[name]

    st_blk_d = EIN("st_blk", [NBLK, 128, MAXNT * 128], bf16)
    stT_blk_d = EIN("stT_blk", [NBLK, 128, MAXNT * 128], bf16)
    if OLDGATHER:
        src32_d = EIN("src32", [128, T], i32)
    else:
        idx16_d = EIN("idx16", [128, NBLK * IDXW], i16)
    ea_sb_d = EIN("ea_sb", [128, T])
    ext11_d = EIN("ext11", [11, T * 128], bf16)
    xt6_own_d = EIN("xt6_own", [5, PADN], bf16)
    W1c_d = EIN("W1c", [11, DIMS[0]], bf16)
    WR1f5_d = EIN("WR1f5", [5, DIMS[0]], bf16)
    WL2_d = EIN("WL2", [DIMS[0], DIMS[1]], bf16)
    WR2_d = EIN("WR2", [DIMS[0], DIMS[1]], bf16)
    WL3_d = EIN("WL3", [DIMS[1], DIMS[2]], bf16)
    WR3_d = EIN("WR3", [DIMS[1], DIMS[2]], bf16)
    weaug_d = [None] + [EIN(f"weaug{i}", [1, NBLK * DIMS[i - 1]], bf16)
                        for i in (2, 3)]
    weRep_d = [EIN(f"weRep{i}", [128, DIMS[i - 1]]) for i in (1, 2, 3)]
    attB_d = [EIN(f"attB{i}", [128, GBS[i - 1] * DIMS[i - 1]], bf16)
              for i in (1, 2, 3)]
    biasRep_d = [EIN(f"biasRep{i}", [128, DIMS[i - 1]]) for i in (1, 2, 3)]
    iota_d = EIN("iota_row", [128, 128], bf16)
    ident_d = EIN("ident", [128, 128], bf16)
    batchloc_d = EIN("batchloc", [128, NBLK], bf16)
    g_rows_d = EIN("g_rows", [128, 1], i32)
    rcnt_d = EIN("rcnt", [128, 4])
    w4rep_d = EIN("w4rep", [128, 8])
    b4_d = EIN("b4v", [128, 1])

    out_d = nc.dram_tensor("out", [B, 1], f32, kind="ExternalOutput")

    # layer-3 table padded to TBW cols so gather rows are 256B
    tables = [None,
              nc.dram_tensor("table2", [NC * PADN, DIMS[1]], bf16),
              nc.dram_tensor("table3", [NC * PADN, TBW], bf16)]
    stagec = [[nc.dram_tensor(f"stage{i}_{k}",
                              [BPC * 128, DIMS[1] if i == 2 else TBW], bf16)
               for k in range(CHUNKS)] for i in (2, 3)]
    pool_full = nc.dram_tensor("pool_full", [POOLPAD, 8], f32)
    pool_red = nc.dram_tensor("pool_red", [B, 8], f32)

    def issue_allgather(li, k):
        tab = tables[li + 1]
        w = tab.shape[1]
        r0 = k * NC * BPC * 128
        r1 = (k + 1) * NC * BPC * 128
        if NOCC:
            nc.sync.dma_start(tab[r0:r0 + BPC * 128, :], stagec[li][k][:])
        else:
            nc.gpsimd.collective_compute(
                "AllGather", mybir.AluOpType.bypass,
                replica_groups=[list(range(NC))],
                ins=[stagec[li][k].ap().opt()],
                outs=[tab.ap()[r0:r1, :].opt()],
            )

    with tile.TileContext(nc) as tc:
        ctx = contextlib.ExitStack()
        with ctx:
            consts = ctx.enter_context(tc.tile_pool(name="consts", bufs=1))
            meta = ctx.enter_context(tc.tile_pool(name="meta", bufs=1))
            xrp = ctx.enter_context(tc.tile_pool(name="xrp", bufs=1))
            stp = ctx.enter_context(tc.tile_pool(name="stp", bufs=4))
            gp = ctx.enter_context(tc.tile_pool(name="gp", bufs=4))
            gpg = ctx.enter_context(tc.tile_pool(name="gpg", bufs=6))
            sp = ctx.enter_context(tc.tile_pool(name="sp", bufs=4))
            ep = ctx.enter_context(tc.tile_pool(name="ep", bufs=3))
            pst = ctx.enter_context(tc.tile_pool(name="psum_t", bufs=2, space="PSUM"))
            psb = ctx.enter_context(tc.tile_pool(name="psum_blk", bufs=2, space="PSUM"))
            pse = ctx.enter_context(tc.tile_pool(name="psum_epi", bufs=1, space="PSUM"))
            chp = ctx.enter_context(tc.tile_pool(name="chunk", bufs=2))

            def load_const(dram, shape, d=f32):
                t = consts.tile(list(shape), d, tag=dram.name + "_c")
                nc.sync.dma_start(t[:], dram[:])
                return t
            iota_t = load_const(iota_d, [128, 128], bf16)
            ident_t = load_const(ident_d, [128, 128], bf16)
            W1c_t = load_const(W1c_d, [11, DIMS[0]], bf16)
            WR1f5_t = load_const(WR1f5_d, [5, DIMS[0]], bf16)
            WL2_t = [consts.tile([128, DIMS[1]], bf16, tag=f"wl2_{k}", name=f"wl2_{k}")
                     for k in range(2)]
            WR2_t = [consts.tile([128, DIMS[1]], bf16, tag=f"wr2_{k}", name=f"wr2_{k}")
                     for k in range(2)]
            for k in range(2):
                nc.sync.dma_start(WL2_t[k][:], WL2_d[k * 128:(k + 1) * 128, :])
                nc.sync.dma_start(WR2_t[k][:], WR2_d[k * 128:(k + 1) * 128, :])
            WL3_t = load_const(WL3_d, [128, DIMS[2]], bf16)
            WR3_t = load_const(WR3_d, [128, DIMS[2]], bf16)
            weRep_t = [load_const(weRep_d[i], [128, DIMS[i]]) for i in range(3)]
            attB_t = [load_const(attB_d[i], [128, GBS[i] * DIMS[i]], bf16)
                      for i in range(3)]
            biasRep_t = [load_const(biasRep_d[i], [128, DIMS[i]]) for i in range(3)]
            batchloc_t = load_const(batchloc_d, [128, NBLK], bf16)
            g_rows_t = load_const(g_rows_d, [128, 1], i32)
            rcnt_t = load_const(rcnt_d, [128, 4])
            w4rep_t = load_const(w4rep_d, [128, 8])
            b4_t = load_const(b4_d, [128, 1])
            if OLDGATHER:
                src_t = meta.tile([128, T], i32)
                nc.sync.dma_start(src_t[:], src32_d[:])
            else:
                idx_t = meta.tile([128, NBLK * IDXW], i16)
                nc.sync.dma_start(idx_t[:], idx16_d[:])
            ea_t = meta.tile([128, T], f32)
            nc.sync.dma_start(ea_t[:], ea_sb_d[:])

            xr_t = [xrp.tile([128, NBLK * DIMS[i]], bf16, tag=f"xr{i}", name=f"xr{i}")
                    for i in range(3)]
            for i in (1, 2):
                nc.sync.dma_start(xr_t[i][127:128, :], weaug_d[i][:])
            # xl of own nodes, kept in SBUF for the self-loop tiles of L2/L3
            # (row 127 zeroed so sblk's ea row contributes nothing); layer-3
            # slab padded to TBW-wide blocks to match the padded table
            xl_t = [None,
                    xrp.tile([128, NBLK * DIMS[1]], bf16, tag="xl1", name="xl1"),
                    xrp.tile([128, NBLK * TBW], bf16, tag="xl2", name="xl2")]
            for i in (1, 2):
                nc.gpsimd.memset(xl_t[i][:], 0.0)

            # ---- preamble: own xr1 (minus bias1: folds +bias into the
            # dst-correction so the L1 epilogue skips the bias add) ----
            CH = 16
            for ch in range(-(-NBLK // CH)):
                j0, j1 = ch * CH, min((ch + 1) * CH, NBLK)
                xchunk = chp.tile([5, CH * 128], bf16, tag="xchunk")
                nc.sync.dma_start(xchunk[:, :(j1 - j0) * 128],
                                  xt6_own_d[:, j0 * 128:j1 * 128])
                for j in range(j1 - j0):
                    b = j0 + j
                    pt = psb.tile([128, DIMS[0]], f32, tag="blk_ps", space="PSUM")
                    nc.tensor.matmul(pt[:], lhsT=xchunk[:, j * 128:(j + 1) * 128],
                                     rhs=WR1f5_t[:], start=True, stop=True)
                    D0 = DIMS[0]
                    nc.vector.tensor_tensor(
                        out=xr_t[0][0:127, b * D0:(b + 1) * D0],
                        in0=pt[0:127, :], in1=biasRep_t[0][0:127, :],
                        op=Alu.subtract)

            # ---- layers ----
            _gq = [0]
            pool_ps = psb.tile([128, 8], f32, tag="pool_ps", space="PSUM", bufs=1)
            for li in range(3):
                H, C = HEADS[li]
                D = DIMS[li]
                W = WIDTHS[li]
                GB = GBS[li]
                PSLOT = max(D, 8)
                table = tables[li]
                is_last = li == 2
                is_first = li == 0
                XLW = DIMS[li] if li < 2 else TBW   # xl_t/table col stride

                def issue_gathers(bb, tt0):
                    nl, nh = ntl[bb], nth[bb]
                    ntb = nt_pb[bb]
                    gtb = gpg.tile([128, (MAXNT - 1) * TBW], bf16,
                                   tag=f"gtb{li}", name=f"gtb{li}")
                    gtb3 = gtb[:, :(ntb - 1) * TBW].rearrange(
                        "p (t d) -> p t d", d=TBW)
                    if OLDGATHER:
                        for t in range(1, ntb):
                            nc.gpsimd.indirect_dma_start(
                                out=gtb3[:, t - 1, :], out_offset=None,
                                in_=table[:],
                                in_offset=IOA(ap=src_t[:, tt0 + t:tt0 + t + 1],
                                              axis=0))
                        return gtb3
                    # SWDGE gather caps at 1024 indices per call;
                    # round-robin the 4 SWDGE queues
                    GMAX = 8
                    regions = [(0, nl, table[:]),
                               (nl, nh, table[LOHALF:NC * PADN, :])]
                    for roff, rnt, rtab in regions:
                        for q0 in range(0, rnt, GMAX):
                            qn = min(GMAX, rnt - q0)
                            o = roff + q0
                            nc.gpsimd.dma_gather(
                                out_ap=gtb3[:, o:o + qn, :],
                                in_ap=rtab,
                                idxs_ap=idx_t[:, bb * IDXW + o * 8:
                                              bb * IDXW + (o + qn) * 8],
                                num_idxs=qn * 128,
                                num_idxs_reg=qn * 128,
                                elem_size=TBW,
                                queue_num=_gq[0] % 4)
                            _gq[0] += 1
                    return gtb3

                def run_epilogue(b, pblk):
                    den = sp.tile([128, H], f32, tag="den")
                    nc.vector.tensor_scalar_add(den[:], pblk[:, D:D + H], 1e-30)
                    rden = sp.tile([128, H], f32, tag="rden")
                    nc.vector.reciprocal(rden[:], den[:])
                    hr = ep.tile([128, D], f32, tag="hr")
                    # num = pblk - xr[dst]*cden - we*cexea; for L1 the psum-based
                    # messages cover all edges (cden = den), for L2/L3 only the
                    # self tile (selfden / selfexea scatter columns)
                    if is_first:
                        cden = den
                        exea_src = pblk[:, D + H:D + 2 * H]
                    else:
                        cden = sp.tile([128, H], f32, tag="sden")
                        nc.vector.tensor_copy(cden[:], pblk[:, D + H:D + 2 * H])
                        exea_src = pblk[:, D + 2 * H:D + 3 * H]
                    exea = sp.tile([128, H], f32, tag="exea")
                    nc.vector.tensor_copy(exea[:], exea_src)
                    meng = nc.gpsimd if is_first else nc.vector
                    meng.tensor_tensor(
                        out=hr[:].rearrange("p (h c) -> p h c", h=H),
                        in0=xr_t[li][:, b * D:(b + 1) * D].rearrange(
                            "p (h c) -> p h c", h=H),
                        in1=cden[:].unsqueeze(2).to_broadcast([128, H, C]),
                        op=Alu.mult)
                    num = ep.tile([128, D], f32, tag="num")
                    nc.vector.tensor_tensor(out=num[:], in0=pblk[:, 0:D],
                                            in1=hr[:], op=Alu.subtract)
                    hr2 = ep.tile([128, D], f32, tag="hr2")
                    meng.tensor_tensor(
                        out=hr2[:].rearrange("p (h c) -> p h c", h=H),
                        in0=weRep_t[li][:].rearrange("p (h c) -> p h c", h=H),
                        in1=exea[:].unsqueeze(2).to_broadcast([128, H, C]),
                        op=Alu.mult)
                    nc.vector.tensor_tensor(out=num[:], in0=num[:],
                                            in1=hr2[:], op=Alu.subtract)
                    nc.vector.tensor_tensor(
                        out=hr[:].rearrange("p (h c) -> p h c", h=H),
                        in0=num[:].rearrange("p (h c) -> p h c", h=H),
                        in1=rden[:].unsqueeze(2).to_broadcast([128, H, C]),
                        op=Alu.mult)
                    if not is_first:
                        nc.vector.tensor_tensor(out=hr[:], in0=hr[:],
                                                in1=biasRep_t[li][:], op=Alu.add)
                    h = ep.tile([128, D], bf16, tag="h_blk")
                    nc.scalar.activation(h[:], hr[:], Act.Tanh)

                    if not is_last:
                        D2 = DIMS[li + 1]
                        XLW2 = DIMS[li + 1] if li + 1 < 2 else TBW
                        WLn = [WL2_t[0], WL2_t[1]] if li == 0 else [WL3_t]
                        WRn = [WR2_t[0], WR2_t[1]] if li == 0 else [WR3_t]
                        nk = D // 128
                        hT = []
                        for k in range(nk):
                            tp = pse.tile([128, 128], bf16, tag="epi_ps", space="PSUM")
                            nc.tensor.transpose(tp[:], h[:, k * 128:(k + 1) * 128],
                                                ident_t[:])
                            hTk = ep.tile([128, 128], bf16, tag=f"hT{k}")
                            nc.vector.tensor_copy(hTk[:], tp[:])
                            hT.append(hTk)
                        pxl = pse.tile([128, D2], f32, tag="epi_ps", space="PSUM")
                        for k in range(nk):
                            nc.tensor.matmul(pxl[:], lhsT=hT[k][:], rhs=WLn[k][:],
                                             start=(k == 0), stop=(k == nk - 1))
                        nc.vector.tensor_copy(
                            xl_t[li + 1][0:127, b * XLW2:b * XLW2 + D2],
                            pxl[0:127, :])
                        kc, bo = divmod(b, BPC)
                        nc.sync.dma_start(
                            stagec[li][kc][bo * 128:(bo + 1) * 128, :],
                            xl_t[li + 1][:, b * XLW2:(b + 1) * XLW2])
                        pxr = pse.tile([128, D2], f32, tag="epi_ps", space="PSUM")
                        for k in range(nk):
                            nc.tensor.matmul(pxr[:], lhsT=hT[k][:], rhs=WRn[k][:],
                                             start=(k == 0), stop=(k == nk - 1))
                        nc.vector.tensor_copy(
                            xr_t[li + 1][0:127, b * D2:(b + 1) * D2], pxr[0:127, :])
                        if bo == BPC - 1:
                            issue_allgather(li, kc)
                    else:
                        Sg = stp.tile([128, 128], bf16, tag="sg_tile")
                        nc.vector.tensor_tensor(
                            out=Sg[:],
                            in0=batchloc_t[:, b:b + 1].to_broadcast([128, 128]),
                            in1=iota_t[:], op=Alu.is_equal)
                        nc.tensor.matmul(pool_ps[:], lhsT=Sg[:], rhs=h[:],
                                         start=(b == 0), stop=(b == NBLK - 1))

                pending = None
                LA = 4
                gtbs = {}
                if not is_first:
                    tt = 0
                    for bb in range(NBLK):
                        if bb < LA:
                            gtbs[bb] = issue_gathers(bb, tt)
                        tt += nt_pb[bb]
                t0 = 0
                tla = sum(nt_pb[:LA]) if NBLK > LA else 0
                for b in range(NBLK):
                    nl, nh = ntl[b], nth[b]
                    nt = nt_pb[b]
                    pblk = psb.tile([128, W], f32, tag="blk_ps", space="PSUM")
                    stT = stp.tile([128, MAXNT * 128], bf16, tag="stT_blk")
                    nc.sync.dma_start(stT[:, :nt * 128],
                                      stT_blk_d[b, :, :nt * 128])
                    if is_first:
                        extb = chp.tile([11, MAXNT * 128], bf16, tag="extb")
                        nc.sync.dma_start(extb[:, :nt * 128],
                                          ext11_d[:, t0 * 128:(t0 + nt) * 128])
                    else:
                        if b + LA < NBLK:
                            gtbs[b + LA] = issue_gathers(b + LA, tla)
                            tla += nt_pb[b + LA]
                        sblk = stp.tile([128, MAXNT * 128], bf16, tag="st_blk")
                        nc.sync.dma_start(sblk[:, :nt * 128],
                                          st_blk_d[b, :, :nt * 128])
                        gtb3 = gtbs.pop(b)
                        # ---- self-loop tile 0: xl is block-local, no gather.
                        # Scatter carries [ex*z | ex | ex | ex*ea]; epilogue
                        # corrections remove the xr and ea*we parts.
                        ptile = pst.tile([128, GB, PSLOT], f32, tag="t_ps",
                                         space="PSUM")
                        nc.tensor.matmul(
                            ptile[:, 0, 0:D], lhsT=sblk[:, 0:128],
                            rhs=xr_t[li][:, b * D:(b + 1) * D],
                            start=True, stop=False)
                        nc.tensor.matmul(
                            ptile[:, 0, 0:D], lhsT=sblk[:, 0:128],
                            rhs=xl_t[li][:, b * XLW:b * XLW + D],
                            start=False, stop=True)
                        up = sp.tile([128, GB * D], bf16, tag="up_t")
                        nc.scalar.activation(
                            up[:, :D].rearrange("p (g d) -> p g d", g=1),
                            ptile[:, 0:1, 0:D], Act.Prelu, alpha=0.2)
                        v = sp.tile([128, GB * D], bf16, tag="v_t")
                        nc.vector.tensor_tensor(
                            out=v[:, :D], in0=up[:, :D],
                            in1=attB_t[li][:, :D], op=Alu.mult)
                        lg = sp.tile([128, GB * H], bf16, tag="lg")
                        with nc.allow_low_precision(reason="bf16 logits"):
                            nc.vector.tensor_reduce(
                                out=lg[:, :H],
                                in_=v[:, :D].rearrange("p (gh c) -> p gh c", c=C),
                                axis=mybir.AxisListType.X, op=Alu.add)
                        yt = gp.tile([128, GB, W], bf16, tag="y_tile")
                        nc.scalar.activation(
                            yt[:, 0:1, D:D + H],
                            lg[:, :H].rearrange("p (g h) -> p g h", g=1),
                            Act.Exp)
                        exv = yt[:, 0:1, D:D + H]
                        nc.vector.tensor_tensor(
                            out=yt[:, 0:1, 0:D].rearrange("p g (h c) -> p g h c",
                                                          h=H),
                            in0=ptile[:, 0:1, 0:D].rearrange(
                                "p g (h c) -> p g h c", h=H),
                            in1=exv.unsqueeze(3).to_broadcast([128, 1, H, C]),
                            op=Alu.mult)
                        nc.scalar.activation(
                            yt[:, 0:1, D + H:D + 2 * H],
                            lg[:, :H].rearrange("p (g h) -> p g h", g=1),
                            Act.Exp)
                        nc.scalar.activation(
                            yt[:, 0:1, D + 2 * H:D + 3 * H], exv,
                            Act.Copy, scale=ea_t[:, t0:t0 + 1])
                        nc.tensor.matmul(pblk[:], lhsT=stT[:, 0:128],
                                         rhs=yt[:, 0, :],
                                         start=True, stop=(nt == 1))
                    for g0 in range(0 if is_first else 1, nt, GB):
                        gs = min(GB, nt - g0)
                        ptile = pst.tile([128, GB, PSLOT], f32, tag="t_ps",
                                         space="PSUM")
                        for i in range(gs):
                            t = g0 + i
                            if is_first:
                                nc.tensor.matmul(
                                    ptile[:, i, 0:D],
                                    lhsT=extb[:, t * 128:(t + 1) * 128],
                                    rhs=W1c_t[:], start=True, stop=True)
                            else:
                                nc.tensor.matmul(
                                    ptile[:, i, 0:D],
                                    lhsT=sblk[:, t * 128:(t + 1) * 128],
                                    rhs=xr_t[li][:, b * D:(b + 1) * D],
                                    start=True, stop=False)
                                nc.tensor.matmul(
                                    ptile[:, i, 0:D],
                                    lhsT=ident_t[:],
                                    rhs=gtb3[:, t - 1, 0:D],
                                    start=False, stop=True)
                        gsl = gtb3[:, g0 - 1:g0 - 1 + gs, 0:D] if not is_first \
                            else None
                        zsrc = ptile[:, 0:gs, 0:D]
                        up = sp.tile([128, GB * D], bf16, tag="up_t")
                        nc.scalar.activation(
                            up[:, :gs * D].rearrange("p (g d) -> p g d", g=gs),
                            zsrc, Act.Prelu, alpha=0.2)
                        v = sp.tile([128, GB * D], bf16, tag="v_t")
                        nc.vector.tensor_tensor(
                            out=v[:, :gs * D], in0=up[:, :gs * D],
                            in1=attB_t[li][:, :gs * D], op=Alu.mult)
                        lg = sp.tile([128, GB * H], bf16, tag="lg")
                        with nc.allow_low_precision(reason="bf16 logits"):
                            nc.vector.tensor_reduce(
                                out=lg[:, :gs * H],
                                in_=v[:, :gs * D].rearrange("p (gh c) -> p gh c",
                                                            c=C),
                                axis=mybir.AxisListType.X, op=Alu.add)
                        yt = gp.tile([128, GB, W], bf16, tag="y_tile")
                        nc.scalar.activation(
                            yt[:, 0:gs, D:D + H],
                            lg[:, :gs * H].rearrange("p (g h) -> p g h", g=gs),
                            Act.Exp)
                        exv = yt[:, 0:gs, D:D + H]
                        nc.vector.tensor_tensor(
                            out=yt[:, 0:gs, 0:D].rearrange("p g (h c) -> p g h c",
                                                           h=H),
                            in0=(zsrc if is_first else gsl).rearrange(
                                     "p g (h c) -> p g h c", h=H),
                            in1=exv.unsqueeze(3).to_broadcast([128, gs, H, C]),
                            op=Alu.mult)
                        if is_first:
                            nc.gpsimd.tensor_tensor(
                                out=yt[:, 0:gs, D + H:D + 2 * H],
                                in0=exv,
                                in1=ea_t[:, t0 + g0:t0 + g0 + gs].unsqueeze(2)
                                    .to_broadcast([128, gs, H]),
                                op=Alu.mult)
                        for i in range(gs):
                            nc.tensor.matmul(
                                pblk[:] if is_first else pblk[:, 0:D + H],
                                lhsT=stT[:, (g0 + i) * 128:(g0 + i + 1) * 128],
                                rhs=yt[:, i, 0:W] if is_first else yt[:, i, 0:D + H],
                                start=(is_first and g0 == 0 and i == 0),
                                stop=(g0 + i == nt - 1))
                    t0 += nt
                    run_epilogue(b, pblk)
            # ---- pooling + head ----
            pool_sb = ep.tile([128, 8], f32, tag="pool_sb")
            nc.vector.tensor_copy(pool_sb[:], pool_ps[:])
            zero8 = consts.tile([128, 8], f32, tag="zero8")
            nc.gpsimd.memset(zero8[:], 0.0)
            for i in range(POOLPAD // 128):
                nc.sync.dma_start(pool_full[i * 128:(i + 1) * 128, :], zero8[:])
            nc.gpsimd.indirect_dma_start(
                out=pool_full[:], out_offset=IOA(ap=g_rows_t[:, :1], axis=0),
                in_=pool_sb[:], in_offset=None)
            if NOCC:
                nc.sync.dma_start(pool_red[:], pool_full[0:B, :])
            else:
                nc.gpsimd.collective_compute(
                    "AllReduce", mybir.AluOpType.add,
                    replica_groups=[list(range(NC))],
                    ins=[pool_full.ap()[0:B, :].opt()], outs=[pool_red.ap().opt()])
            for i in range(B // 128):
                pt = ep.tile([128, 8], f32, tag="head_in")
                nc.sync.dma_start(pt[:], pool_red[i * 128:(i + 1) * 128, :])
                pw = ep.tile([128, 8], f32, tag="head_w")
                nc.vector.tensor_tensor(out=pw[:], in0=pt[:], in1=w4rep_t[:],
                                        op=Alu.mult)
                hred = ep.tile([128, 1], f32, tag="head_red")
                nc.vector.tensor_reduce(out=hred[:], in_=pw[:],
                                        axis=mybir.AxisListType.X, op=Alu.add)
                nc.vector.tensor_tensor(out=hred[:], in0=hred[:],
                                        in1=rcnt_t[:, i:i + 1], op=Alu.mult)
                nc.vector.tensor_tensor(out=hred[:], in0=hred[:], in1=b4_t[:],
                                        op=Alu.add)
                nc.sync.dma_start(out_d[i * 128:(i + 1) * 128, :], hred[:])

    nc.compile()
    nc._kernel_input_names = set(ein)
    return nc


def _get_program(inputs):
    import os
    variant = os.environ.get("KVARIANT", "")
    pre = _host_preprocess(inputs["x"], inputs["edge_index"], inputs["edge_attr"],
                           inputs["batch"])
    key = (pre["ntl"], pre["nth"], variant)
    if key not in _CACHE:
        _CACHE[key] = _build_program(pre["ntl"], pre["nth"], pre["T"],
                                     variant=variant)
    return _CACHE[key], pre


def _make_in_maps(inputs, pre):
    import ml_dtypes
    bf16 = ml_dtypes.bfloat16
    wts = _host_weights(inputs)
    xt6_own = _build_x_inputs(inputs["x"])
    iota = np.tile(np.arange(128, dtype=np.float32), (128, 1))
    ident = np.eye(128, dtype=np.float32)
    in_maps = []
    for c in range(NC):
        m = dict(
            st_blk=pre["st_blk"][c].astype(bf16),
            stT_blk=pre["stT_blk"][c].astype(bf16),
            idx16=pre["idx16"][c],
            src32=pre["src32"][c],
            ea_sb=pre["ea_sb"][c],
            ext11=pre["ext11"][c].astype(bf16),
            xt6_own=xt6_own[c].astype(bf16),
            W1c=wts["W1c"].astype(bf16), WR1f5=wts["WR1f5"].astype(bf16),
            WL2=wts["WL2"].astype(bf16), WR2=wts["WR2"].astype(bf16),
            WL3=wts["WL3"].astype(bf16), WR3=wts["WR3"].astype(bf16),
            weRep1=wts["weRep1"], weRep2=wts["weRep2"], weRep3=wts["weRep3"],
            iota_row=iota.astype(bf16), ident=ident.astype(bf16),
            batchloc=pre["batchloc"][c].astype(bf16), g_rows=pre["g_rows"][c],
            rcnt=np.ascontiguousarray(pre["rcnt"].reshape(4, 128).T),
            w4rep=wts["w4rep"], b4v=np.full((128, 1), wts["b4"], np.float32),
        )
        for i in (1, 2, 3):
            m[f"weaug{i}"] = wts[f"weaug{i}"].astype(bf16)
            m[f"attB{i}"] = wts[f"attB{i}"].astype(bf16)
            m[f"biasRep{i}"] = wts[f"biasRep{i}"]
        in_maps.append(m)
    return in_maps


def kernel(**inputs):
    from concourse.bass_utils import run_bass_kernel_spmd
    nc, pre = _get_program(inputs)
    in_maps = _make_in_maps(inputs, pre)
    names = getattr(nc, "_kernel_input_names", None)
    if names is not None:
        in_maps = [{k: v for k, v in m.items() if k in names} for m in in_maps]
    res = run_bass_kernel_spmd(nc, in_maps, core_ids=list(range(NC)))
    return np.asarray(res.results[0]["out"], np.float32)
